# revision 8
# baseline (speedup 1.0000x reference)
"""Trainium2 Bass kernel for nn_GamePhysicsModule (gnn_message_passing).

Strategy
--------
reference computes, per batch b of 4:
  * per-object: p = relu(x@pp_w1+b)@pp_w2+b  -> 4 sigmoid heads (only cols 0..3
    of pp_w2 are observable), vel/frc heads (x@[512,3]).
  * pairwise collision MLP over the Np = N(N-1)/2 upper-triangle pairs:
      h1 = relu([x_i | x_j] @ cd_w1 + b1) = relu(A_i + B_j + b1)
    with A = x@W1a, B = x@W1b (cd_w1 split in halves) -- the big win: the
    1024x256 layer-1 matmul collapses to two 512x256 matmuls over N=256
    objects instead of Np=32640 pairs.

Pair sharding: pairs grouped by diagonal d = j - i (d = 1..255).  Diagonal d
is a contiguous sliding-window sum: h1T[:, i] = A_T[:, i] + B_T[:, i+d].
8 cores = 4 batches x 2.  Within a batch the two cores split every diagonal
at the anti-diagonal (i + j < 255 vs >= 255); the second core runs the SAME
program on reflected inputs (x rows reversed, W1a/W1b swapped), which maps
its half onto windows starting at i'=0.  Per-object work is sharded evenly
(128 of the 1024 (b, n) rows per core).

On-device per core:
  A: P_T/Q_T = WP.T @ x_T, WQ.T @ x_T            ([256f', 256obj] each)
  B: for each group of D consecutive diagonals (fixed window W, D*W <= 512):
       h1 = relu(P win + Q sliding win)          (DVE add + ACT/DVE relu)
       h2 = relu(W2.T @ h1 + b2)                 (PE fp32r + ACT evac)
       prob = sigmoid(w3.T @ h2 + b3)            (PE M=1 + ACT)
  C: per-object MLP + heads for this core's 128 rows.
Host: scatter probs into the symmetric collision matrix, stack head slices.
"""

import os
import sys

import numpy as np

for _p in ("/opt/trn_rl_repo", "/root/.axon_site/_ro/pypackages"):
    if os.path.isdir(_p) and _p not in sys.path:
        sys.path.insert(0, _p)

B, N, H = 4, 256, 512
PART = 128
FD = 512          # max free-dim per pair-group (one PSUM bank of fp32)
NCORES = 8

# dtype knobs: "f32r" = float32r matmuls (full-rate PE, ~tf32 precision on HW)
MM_DTYPE = os.environ.get("GP_MM_DTYPE", "f32r")
H1_DTYPE = os.environ.get("GP_H1_DTYPE", "f32")   # h1/h2 storage dtype


def _groups():
    """Diagonal groups: (d0, D, W). Group g covers diagonals d0..d0+D-1,
    each with a fixed window of W pair-slots (valid length <= W)."""
    gs = []
    d0 = 1
    while d0 <= N - 1:
        W = (N - d0 + 1) // 2            # ceil((N - d0) / 2) = max valid len
        W += W & 1                       # fp32r ISA: innermost count must be even
        D = max(1, min(FD // W, N - d0))
        gs.append((d0, D, W))
        d0 += D
    return gs


_GROUPS = _groups()
NG = len(_GROUPS)
QPAD = max(0, max(d0 + D - 1 + W - 1 - (N - 1) for d0, D, W in _GROUPS))


def _scatter_idx():
    """Per reflection flag: (pos into flat probs, i coord, j coord)."""
    out = {}
    for refl in (0, 1):
        pos, ii, jj = [], [], []
        for g, (d0, D, W) in enumerate(_GROUPS):
            for l in range(D):
                d = d0 + l
                le = (N - d) // 2          # even-core (refl=0) valid count
                lo = (N - d) - le          # odd-core  (refl=1) valid count
                L = le if refl == 0 else lo
                if L <= 0:
                    continue
                i = np.arange(L)
                pos.append(g * FD + l * W + i)
                if refl == 0:
                    ii.append(i)
                    jj.append(i + d)
                else:
                    ii.append(N - 1 - i - d)
                    jj.append(N - 1 - i)
        out[refl] = (
            np.concatenate(pos),
            np.concatenate(ii),
            np.concatenate(jj),
        )
    return out


_SCATTER = _scatter_idx()


def _build_nc(loop_k=None):
    import concourse.bacc as bacc
    import concourse.bass as bass
    import concourse.tile as tile
    from concourse import mybir

    f32 = mybir.dt.float32
    dt_mm = mybir.dt.float32r if MM_DTYPE == "f32r" else f32
    dt_h1 = mybir.dt.bfloat16 if H1_DTYPE == "bf16" else f32
    AF = mybir.ActivationFunctionType
    Alu = mybir.AluOpType

    nc = bacc.Bacc()

    xt = nc.declare_dram_parameter("xt", [H, N], dt_mm, False)
    wp = nc.declare_dram_parameter("wp", [H, N], dt_mm, False)
    wq = nc.declare_dram_parameter("wq", [H, N], dt_mm, False)
    cb1 = nc.declare_dram_parameter("cb1", [PART, 2], f32, False)
    w2 = nc.declare_dram_parameter("w2", [2 * PART, PART], dt_mm, False)
    cb2 = nc.declare_dram_parameter("cb2", [PART, 1], f32, False)
    w3 = nc.declare_dram_parameter("w3", [PART, 2], dt_mm, False)
    cb3 = nc.declare_dram_parameter("cb3", [1, 1], f32, False)
    xr = nc.declare_dram_parameter("xr", [H, PART], dt_mm, False)
    pw1 = nc.declare_dram_parameter("pw1", [H, H], dt_mm, False)
    pb1 = nc.declare_dram_parameter("pb1", [PART, 4], f32, False)
    pw2c = nc.declare_dram_parameter("pw2c", [H, 4], dt_mm, False)
    pb2c = nc.declare_dram_parameter("pb2c", [4, 1], f32, False)
    vfw = nc.declare_dram_parameter("vfw", [H, 6], dt_mm, False)
    vfb = nc.declare_dram_parameter("vfb", [6, 1], f32, False)
    scl = nc.declare_dram_parameter("scl", [4, 1], f32, False)

    probs = nc.declare_dram_parameter("probs", [NG, FD], f32, True)
    heads = nc.declare_dram_parameter("heads", [4, PART], f32, True)
    vf = nc.declare_dram_parameter("vf", [6, PART], f32, True)

    def mm(ap):
        return ap

    import contextlib

    with tile.TileContext(nc) as tc:
        loop_cm = (
            tc.For_i(0, loop_k, 1, hint_engines=tuple(nc.engines.keys()))
            if loop_k
            else contextlib.nullcontext()
        )
        with loop_cm:
         with (
            tc.tile_pool(name="cst", bufs=1) as cst,
            tc.tile_pool(name="wrk", bufs=3) as wrk,
            tc.tile_pool(name="h2w", bufs=3) as h2w,
            tc.tile_pool(name="psA", bufs=2, space="PSUM") as psA,
            tc.tile_pool(name="ps2", bufs=2, space="PSUM") as ps2,
            tc.tile_pool(name="ps3", bufs=2, space="PSUM") as ps3,
        ):
            # ---- load pair-stage constants ----
            xt_sb = cst.tile([PART, 4, N], dt_mm, tag="xt")
            nc.sync.dma_start(
                out=xt_sb, in_=xt[:].rearrange("(c p) i -> p c i", p=PART)
            )
            wp_sb = cst.tile([PART, 4, N], dt_mm, tag="wp")
            nc.sync.dma_start(
                out=wp_sb, in_=wp[:].rearrange("(c p) f -> p c f", p=PART)
            )
            wq_sb = cst.tile([PART, 4, N], dt_mm, tag="wq")
            nc.sync.dma_start(
                out=wq_sb, in_=wq[:].rearrange("(c p) f -> p c f", p=PART)
            )
            w2_sb = cst.tile([PART, 2, PART], dt_mm, tag="w2")
            nc.sync.dma_start(
                out=w2_sb, in_=w2[:].rearrange("(c p) k -> p c k", p=PART)
            )
            w3_sb = cst.tile([PART, 2], dt_mm, tag="w3")
            nc.sync.dma_start(out=w3_sb, in_=w3[:])
            cb1_sb = cst.tile([PART, 2], f32, tag="cb1")
            nc.sync.dma_start(out=cb1_sb, in_=cb1[:])
            cb2_sb = cst.tile([PART, 1], f32, tag="cb2")
            nc.sync.dma_start(out=cb2_sb, in_=cb2[:])
            cb3_sb = cst.tile([1, 1], f32, tag="cb3")
            nc.sync.dma_start(out=cb3_sb, in_=cb3[:])

            # ---- stage A: P_T / Q_T ----
            p_t = cst.tile([PART, 2, N], dt_h1, tag="p_t")
            q_t = cst.tile([PART, 2, N + QPAD], dt_h1, tag="q_t")
            nc.vector.memset(q_t, 0.0)
            for fo in range(2):
                pp = psA.tile([PART, N], f32, tag="psa")
                for hc in range(4):
                    nc.tensor.matmul(
                        pp,
                        mm(wp_sb[:, hc, fo * PART : (fo + 1) * PART]),
                        mm(xt_sb[:, hc, :]),
                        start=(hc == 0),
                        stop=(hc == 3),
                    )
                # fold b1 into the P side
                nc.scalar.activation(
                    p_t[:, fo, :], pp, AF.Identity, bias=cb1_sb[:, fo : fo + 1]
                )
                pq = psA.tile([PART, N], f32, tag="psa")
                for hc in range(4):
                    nc.tensor.matmul(
                        pq,
                        mm(wq_sb[:, hc, fo * PART : (fo + 1) * PART]),
                        mm(xt_sb[:, hc, :]),
                        start=(hc == 0),
                        stop=(hc == 3),
                    )
                nc.vector.tensor_copy(q_t[:, fo, 0:N], pq)

            # ---- stage B: diagonal pair groups ----
            for g, (d0, D, W) in enumerate(_GROUPS):
                FW = D * W
                ht = wrk.tile([PART, 2, FW], dt_mm if dt_h1 is f32 else dt_h1, tag="ht")
                for fo in range(2):
                    in0 = bass.AP(
                        tensor=p_t.tensor,
                        offset=p_t.offset + fo * N,
                        ap=[p_t.ap[0], [0, D], [1, W]],
                    )
                    in1 = bass.AP(
                        tensor=q_t.tensor,
                        offset=q_t.offset + fo * (N + QPAD) + d0,
                        ap=[q_t.ap[0], [1, D], [1, W]],
                    )
                    out3d = ht[:, fo, :].rearrange("p (d w) -> p d w", d=D)
                    nc.vector.tensor_tensor(
                        out=out3d, in0=in0, in1=in1, op=Alu.add
                    )
                    if fo == 0:
                        nc.scalar.activation(ht[:, fo, :], ht[:, fo, :], AF.Relu)
                    else:
                        nc.gpsimd.tensor_scalar(
                            ht[:, fo, :], ht[:, fo, :], 0.0, None, op0=Alu.max
                        )
                ps2t = ps2.tile([PART, FW], f32, tag="ps2")
                for fo in range(2):
                    nc.tensor.matmul(
                        ps2t,
                        mm(w2_sb[:, fo, :]),
                        mm(ht[:, fo, :]),
                        start=(fo == 0),
                        stop=(fo == 1),
                    )
                h2t = h2w.tile([PART, FW], dt_mm if dt_h1 is f32 else dt_h1, tag="h2t")
                nc.vector.tensor_scalar(
                    h2t, ps2t, cb2_sb, 0.0, op0=Alu.add, op1=Alu.max
                )
                ps3t = ps3.tile([2, FW], f32, tag="ps3")
                nc.tensor.matmul(
                    ps3t, mm(w3_sb[:]), mm(h2t[:]), start=True, stop=True
                )
                pr = wrk.tile([1, FW], f32, tag="pr")
                nc.scalar.activation(pr, ps3t[0:1, :], AF.Sigmoid, bias=cb3_sb[:1])
                nc.sync.dma_start(out=probs[g : g + 1, 0:FW], in_=pr)

            # ---- stage C: per-object MLP + heads ----
            xr_sb = cst.tile([PART, 4, PART], dt_mm, tag="xr")
            nc.sync.dma_start(
                out=xr_sb, in_=xr[:].rearrange("(c p) r -> p c r", p=PART)
            )
            pw1_sb = cst.tile([PART, 4, H], dt_mm, tag="pw1")
            nc.sync.dma_start(
                out=pw1_sb, in_=pw1[:].rearrange("(c p) f -> p c f", p=PART)
            )
            pb1_sb = cst.tile([PART, 4], f32, tag="pb1")
            nc.sync.dma_start(out=pb1_sb, in_=pb1[:])
            pw2c_sb = cst.tile([PART, 4, 4], dt_mm, tag="pw2c")
            nc.sync.dma_start(
                out=pw2c_sb, in_=pw2c[:].rearrange("(c p) k -> p c k", p=PART)
            )
            pb2c_sb = cst.tile([4, 1], f32, tag="pb2c")
            nc.sync.dma_start(out=pb2c_sb, in_=pb2c[:])
            vfw_sb = cst.tile([PART, 4, 6], dt_mm, tag="vfw")
            nc.sync.dma_start(
                out=vfw_sb, in_=vfw[:].rearrange("(c p) k -> p c k", p=PART)
            )
            vfb_sb = cst.tile([6, 1], f32, tag="vfb")
            nc.sync.dma_start(out=vfb_sb, in_=vfb[:])
            scl_sb = cst.tile([4, 1], f32, tag="scl")
            nc.sync.dma_start(out=scl_sb, in_=scl[:])

            y1_sb = cst.tile([PART, 4, PART], dt_mm, tag="y1")
            for fo in range(4):
                py = psA.tile([PART, PART], f32, tag="psa")
                for hc in range(4):
                    nc.tensor.matmul(
                        py,
                        mm(pw1_sb[:, hc, fo * PART : (fo + 1) * PART]),
                        mm(xr_sb[:, hc, :]),
                        start=(hc == 0),
                        stop=(hc == 3),
                    )
                nc.scalar.activation(
                    y1_sb[:, fo, :], py, AF.Relu, bias=pb1_sb[:, fo : fo + 1]
                )
            p4 = psA.tile([4, PART], f32, tag="psa")
            for fo in range(4):
                nc.tensor.matmul(
                    p4,
                    mm(pw2c_sb[:, fo, :]),
                    mm(y1_sb[:, fo, :]),
                    start=(fo == 0),
                    stop=(fo == 3),
                )
            sig4 = wrk.tile([4, PART], f32, tag="sig4")
            nc.scalar.activation(sig4, p4, AF.Sigmoid, bias=pb2c_sb[:])
            hd = wrk.tile([4, PART], f32, tag="hd")
            nc.vector.tensor_scalar(hd, sig4, scl_sb[:], None, op0=Alu.mult)
            nc.sync.dma_start(out=heads[:], in_=hd)

            pvf = psA.tile([6, PART], f32, tag="psa")
            for hc in range(4):
                nc.tensor.matmul(
                    pvf,
                    mm(vfw_sb[:, hc, :]),
                    mm(xr_sb[:, hc, :]),
                    start=(hc == 0),
                    stop=(hc == 3),
                )
            vf_sb = wrk.tile([6, PART], f32, tag="vf")
            nc.scalar.activation(vf_sb, pvf, AF.Identity, bias=vfb_sb[:])
            nc.sync.dma_start(out=vf[:], in_=vf_sb)

    nc.compile()
    return nc


def _in_maps(inputs):
    f = lambda a: np.ascontiguousarray(np.asarray(a), dtype=np.float32)
    x = f(inputs["object_features"])
    cd_w1 = f(inputs["cd_w1"])
    w1a, w1b = cd_w1[:H], cd_w1[H:]
    shared = {
        "cb1": f(inputs["cd_b1"]).reshape(2, PART).T,
        "w2": f(inputs["cd_w2"]),
        "cb2": f(inputs["cd_b2"]).reshape(PART, 1),
        "w3": np.concatenate([f(inputs["cd_w3"]), np.zeros((PART, 1), np.float32)], axis=1),
        "cb3": f(inputs["cd_b3"]).reshape(1, 1),
        "pw1": f(inputs["pp_w1"]),
        "pb1": f(inputs["pp_b1"]).reshape(4, PART).T,
        "pw2c": f(inputs["pp_w2"])[:, :4],
        "pb2c": f(inputs["pp_b2"])[:4].reshape(4, 1),
        "vfw": np.concatenate([f(inputs["vel_w"]), f(inputs["frc_w"])], axis=1),
        "vfb": np.concatenate(
            [f(inputs["vel_b"]), f(inputs["frc_b"])]
        ).reshape(6, 1),
        "scl": np.array([[100.0], [1.0], [1.0], [10.0]], np.float32),
    }
    shared = {k: np.ascontiguousarray(v) for k, v in shared.items()}
    x2d = x.reshape(B * N, H)
    maps = []
    for c in range(NCORES):
        b, refl = divmod(c, 2)
        xb = x[b] if refl == 0 else x[b, ::-1]
        m = dict(shared)
        m["xt"] = np.ascontiguousarray(xb.T)
        m["wp"] = w1a if refl == 0 else w1b
        m["wq"] = w1b if refl == 0 else w1a
        m["xr"] = np.ascontiguousarray(x2d[c * PART : (c + 1) * PART].T)
        maps.append(m)
    return maps


_RUNNER = {}


def _get_runner(loop_k=None):
    """Build the program once and return a cached callable
    in_maps -> list of per-core output dicts (jit kept warm across calls)."""
    if loop_k in _RUNNER:
        return _RUNNER[loop_k]

    import jax
    from jax.experimental.shard_map import shard_map
    from jax.sharding import Mesh, PartitionSpec

    from concourse import bass2jax, mybir
    from concourse.bass2jax import _bass_exec_p, install_neuronx_cc_hook

    nc = _build_nc(loop_k)
    install_neuronx_cc_hook()

    part_name = nc.partition_id_tensor.name if nc.partition_id_tensor else None
    in_names, out_names, out_avals, zero_outs = [], [], [], []
    for alloc in nc.m.functions[0].allocations:
        if not isinstance(alloc, mybir.MemoryLocationSet):
            continue
        name = alloc.memorylocations[0].name
        if alloc.kind == "ExternalInput":
            if name != part_name:
                in_names.append(name)
        elif alloc.kind == "ExternalOutput":
            shape = tuple(alloc.tensor_shape)
            dtype = mybir.dt.np(alloc.dtype)
            out_names.append(name)
            out_avals.append(jax.core.ShapedArray(shape, dtype))
            zero_outs.append(np.zeros(shape, dtype))
    n_params = len(in_names)
    all_in = in_names + out_names
    if part_name is not None:
        all_in = all_in + [part_name]

    def _body(*args):
        operands = list(args)
        if part_name is not None:
            operands.append(bass2jax.partition_id_tensor())
        outs = _bass_exec_p.bind(
            *operands,
            out_avals=tuple(out_avals),
            in_names=tuple(all_in),
            out_names=tuple(out_names),
            lowering_input_output_aliases=(),
            sim_require_finite=True,
            sim_require_nnan=True,
            nc=nc,
        )
        return tuple(outs)

    devices = jax.devices()[:NCORES]
    mesh = Mesh(np.asarray(devices), ("core",))
    n_outs = len(out_names)
    sharded = jax.jit(
        shard_map(
            _body,
            mesh=mesh,
            in_specs=(PartitionSpec("core"),) * (n_params + n_outs),
            out_specs=(PartitionSpec("core"),) * n_outs,
            check_rep=False,
        ),
        donate_argnums=tuple(range(n_params, n_params + n_outs)),
        keep_unused=True,
    )

    def run(in_maps):
        concat_in = [
            np.concatenate([m[name] for m in in_maps], axis=0)
            for name in in_names
        ]
        concat_zeros = [
            np.zeros((NCORES * z.shape[0], *z.shape[1:]), z.dtype)
            for z in zero_outs
        ]
        out_arrs = sharded(*concat_in, *concat_zeros)
        return [
            {
                name: np.asarray(out_arrs[i]).reshape(
                    NCORES, *out_avals[i].shape
                )[c]
                for i, name in enumerate(out_names)
            }
            for c in range(NCORES)
        ]

    _RUNNER[loop_k] = run
    return _RUNNER[loop_k]


def _assemble(results):
    mass = np.empty((B * N, 1), np.float32)
    fric = np.empty((B * N, 1), np.float32)
    elas = np.empty((B * N, 1), np.float32)
    dens = np.empty((B * N, 1), np.float32)
    velo = np.empty((B * N, 3), np.float32)
    forc = np.empty((B * N, 3), np.float32)
    coll = np.zeros((B, N, N), np.float32)
    for c in range(NCORES):
        r = results[c]
        sl = slice(c * PART, (c + 1) * PART)
        hd = r["heads"]
        mass[sl, 0] = hd[0]
        fric[sl, 0] = hd[1]
        elas[sl, 0] = hd[2]
        dens[sl, 0] = hd[3]
        velo[sl] = r["vf"][0:3].T
        forc[sl] = r["vf"][3:6].T
        b, refl = divmod(c, 2)
        pos, ii, jj = _SCATTER[refl]
        pv = r["probs"].reshape(-1)[pos]
        coll[b, ii, jj] = pv
        coll[b, jj, ii] = pv
    shp = (B, N, 1)
    return (
        mass.reshape(shp),
        fric.reshape(shp),
        elas.reshape(shp),
        dens.reshape(shp),
        velo.reshape(B, N, 3),
        forc.reshape(B, N, 3),
        coll,
    )


def kernel(**inputs):
    run = _get_runner()
    results = run(_in_maps(inputs))
    return _assemble(results)
